# revision 1
# baseline (speedup 1.0000x reference)
"""Trainium2 Bass kernel for reparameterized-Gaussian linear layer.

Computes: out = input @ (mu + softplus(rho) * eps).T + bias
  input [4096, 2048] f32, mu/rho/eps [2048, 2048] f32, bias [2048] f32
  -> out [4096, 2048] f32

Sharding over 8 cores: 2D grid (t=2 token shards x o=4 out-feature shards).
Each core reads input rows [t*2048:(t+1)*2048] and weight rows
[o*512:(o+1)*512], computes a [2048, 512] output block. This minimizes
per-core HBM traffic (16MB x + 12MB weights + 4MB out = 32MB/core).

Per-core kernel:
  1. w = mu + softplus(rho) * eps  (ACT softplus, DVE mul/add), then
     PE-transpose w into wT [k_part, OUT] resident in SBUF.
  2. Stream 16 token tiles [128, 2048]: PE-transpose each into
     xT [k_part, tok], then 16 accumulating matmuls (float32r fast path)
     into PSUM [128 tok, 512 out], add broadcast bias, DMA out.
"""

import numpy as np

import concourse.bass as bass
import concourse.mybir as mybir
import concourse.tile as tile
from concourse import bacc
from concourse.bass_utils import run_bass_kernel_spmd
from concourse.masks import make_identity

P = 128
N_FULL = 4096
K = 2048
OUT_FULL = 2048
T_SHARDS = 2
O_SHARDS = 4
TOK = N_FULL // T_SHARDS   # 2048 tokens per core
OUT = OUT_FULL // O_SHARDS  # 512 out features per core
KT = K // P                 # 16 contraction tiles
TOKT = TOK // P             # 16 token tiles
OT = OUT // P               # 4 out-feature partition tiles

F32 = mybir.dt.float32
F32R = mybir.dt.float32r

_CACHE = {}


def _build_nc(tok=TOK, k=K, outf=OUT, psum_mm_bufs=2):
    kt_n = k // P
    tokt_n = tok // P
    ot_n = outf // P
    nc = bacc.Bacc(
        "TRN2",
        target_bir_lowering=False,
        debug=False,
        enable_asserts=False,
        num_devices=8,
    )
    x = nc.dram_tensor("x", [tok, k], F32, kind="ExternalInput").ap()
    mu = nc.dram_tensor("mu", [outf, k], F32, kind="ExternalInput").ap()
    rho = nc.dram_tensor("rho", [outf, k], F32, kind="ExternalInput").ap()
    eps = nc.dram_tensor("eps", [outf, k], F32, kind="ExternalInput").ap()
    bias = nc.dram_tensor("bias", [1, outf], F32, kind="ExternalInput").ap()
    out = nc.dram_tensor("out", [tok, outf], F32, kind="ExternalOutput").ap()

    with tile.TileContext(nc) as tc:
        with (
            tc.tile_pool(name="const", bufs=1) as const,
            tc.tile_pool(name="wt", bufs=1) as wtp,
            tc.tile_pool(name="wcomp", bufs=2) as wcomp,
            tc.tile_pool(name="xin", bufs=3) as xin,
            tc.tile_pool(name="xt", bufs=3) as xtp,
            tc.tile_pool(name="psum_t", bufs=4, space="PSUM") as psum_t,
            tc.tile_pool(name="psum_mm", bufs=psum_mm_bufs, space="PSUM") as psum_mm,
            tc.tile_pool(name="outp", bufs=3) as outp,
        ):
            identity = const.tile([P, P], F32)
            make_identity(nc, identity)

            bias_bc = const.tile([P, outf], F32)
            nc.sync.dma_start(bias_bc[:], bias.to_broadcast([P, outf]))

            # Stage A: w = mu + softplus(rho) * eps, transposed into wT.
            wT = wtp.tile([P, kt_n, outf], F32R)  # [k_part, k_outer, out]
            for ot in range(ot_n):
                osl = slice(ot * P, (ot + 1) * P)
                mu_t = wcomp.tile([P, k], F32, tag="mu")
                rho_t = wcomp.tile([P, k], F32, tag="rho")
                eps_t = wcomp.tile([P, k], F32, tag="eps")
                nc.sync.dma_start(mu_t[:], mu[osl, :])
                nc.sync.dma_start(rho_t[:], rho[osl, :])
                nc.sync.dma_start(eps_t[:], eps[osl, :])
                # softplus(rho) = ln(exp(rho) + 1); rho <= ~0.5 here so exp
                # cannot overflow. Exp and Ln share one ACT table.
                sp_t = wcomp.tile([P, k], F32, tag="sp")
                nc.scalar.activation(
                    sp_t[:], rho_t[:], mybir.ActivationFunctionType.Exp
                )
                nc.scalar.activation(
                    sp_t[:], sp_t[:], mybir.ActivationFunctionType.Ln, bias=1.0
                )
                w_t = wcomp.tile([P, k], F32, tag="w")
                nc.vector.tensor_mul(w_t[:], sp_t[:], eps_t[:])
                nc.vector.tensor_add(w_t[:], w_t[:], mu_t[:])
                for kt in range(kt_n):
                    pt = psum_t.tile([P, P], F32)
                    nc.tensor.transpose(
                        pt[:], w_t[:, kt * P : (kt + 1) * P], identity[:]
                    )
                    nc.any.tensor_copy(wT[:, kt, osl], pt[:])

            # Stage B: stream token tiles.
            for tt in range(tokt_n):
                tsl = slice(tt * P, (tt + 1) * P)
                x_t = xin.tile([P, k], F32)
                nc.sync.dma_start(x_t[:], x[tsl, :])
                xT = xtp.tile([P, kt_n, P], F32R)  # [k_part, k_outer, tok]
                for kt in range(kt_n):
                    pt = psum_t.tile([P, P], F32)
                    nc.tensor.transpose(
                        pt[:], x_t[:, kt * P : (kt + 1) * P], identity[:]
                    )
                    nc.any.tensor_copy(xT[:, kt, :], pt[:])
                pmm = psum_mm.tile([P, outf], F32)
                for kt in range(kt_n):
                    nc.tensor.matmul(
                        pmm[:],
                        lhsT=xT[:, kt, :],
                        rhs=wT[:, kt, :],
                        start=(kt == 0),
                        stop=(kt == kt_n - 1),
                    )
                o_t = outp.tile([P, outf], F32)
                nc.vector.tensor_add(o_t[:], pmm[:], bias_bc[:])
                nc.sync.dma_start(out[tsl, :], o_t[:])

    nc.compile()
    return nc


def _get_nc():
    if "nc" not in _CACHE:
        _CACHE["nc"] = _build_nc()
    return _CACHE["nc"]


def _make_in_maps(input, weight_mu, weight_rho, eps_weight, bias):
    in_maps = []
    for core in range(8):
        t, o = divmod(core, O_SHARDS)
        tsl = slice(t * TOK, (t + 1) * TOK)
        osl = slice(o * OUT, (o + 1) * OUT)
        in_maps.append(
            {
                "x": np.ascontiguousarray(input[tsl, :], dtype=np.float32),
                "mu": np.ascontiguousarray(weight_mu[osl, :], dtype=np.float32),
                "rho": np.ascontiguousarray(weight_rho[osl, :], dtype=np.float32),
                "eps": np.ascontiguousarray(eps_weight[osl, :], dtype=np.float32),
                "bias": np.ascontiguousarray(
                    bias[osl].reshape(1, OUT), dtype=np.float32
                ),
            }
        )
    return in_maps


def run_sharded(input, weight_mu, weight_rho, eps_weight, bias, **run_kwargs):
    """Run the SPMD kernel; returns (full_output, BassKernelResults)."""
    nc = _get_nc()
    in_maps = _make_in_maps(input, weight_mu, weight_rho, eps_weight, bias)
    res = run_bass_kernel_spmd(nc, in_maps, list(range(8)), **run_kwargs)
    full = np.empty((N_FULL, OUT_FULL), dtype=np.float32)
    for core in range(8):
        t, o = divmod(core, O_SHARDS)
        full[t * TOK : (t + 1) * TOK, o * OUT : (o + 1) * OUT] = res.results[
            core
        ]["out"]
    return full, res


def kernel(input, weight_mu, weight_rho, eps_weight, bias):
    full, _ = run_sharded(
        np.asarray(input),
        np.asarray(weight_mu),
        np.asarray(weight_rho),
        np.asarray(eps_weight),
        np.asarray(bias),
    )
    return full



# revision 8
# speedup vs baseline: 1.3325x; 1.3325x over previous
"""Trainium2 Bass kernel for reparameterized-Gaussian linear layer.

Computes: out = input @ (mu + softplus(rho) * eps).T + bias
  input [4096, 2048] f32, mu/rho/eps [2048, 2048] f32, bias [2048] f32
  -> out [4096, 2048] f32

Sharding over 8 cores: 2D grid (t=2 token shards x o=4 out-feature shards).
Each core handles input rows [t*2048:(t+1)*2048] and weight rows
[o*512:(o+1)*512], computing a [2048, 512] output block (stored
transposed as [512, 2048]; the host transposes back).

All inputs are cast to bf16 on the host (halves HBM traffic), well
within the 2e-2 rel-err budget.

Per-core kernel:
  1. XBAR DMA transpose-loads bring muT/rhoT/epsT [128k, 4oc, 16kt, 128o]
     and xT blocks [128k, 16kt, 512tok] into SBUF already k-major — no PE
     transposes, no PSUM staging.
  2. wT = muT + softplus(rhoT) * epsT with softplus = ln(1+exp(.)):
     all 4 Exp chunks run before any Ln chunk (dep-enforced) so the ACT
     table set switches exactly twice.
  3. 4 x-blocks x 4 o-chunks x 16 k-tiles of bf16 matmuls accumulate
     outT chunks [128o, 512tok] in PSUM; the PSUM->SBUF copy runs on ACT
     as Identity with the per-partition bias folded in; out DMA as bf16.
"""

import ml_dtypes
import numpy as np

import concourse.bass as bass
import concourse.mybir as mybir
import concourse.tile as tile
from concourse import bacc
from concourse.bass_utils import run_bass_kernel_spmd
from concourse.tile import add_dep_helper

P = 128
N_FULL = 4096
K = 2048
OUT_FULL = 2048
T_SHARDS = 2
O_SHARDS = 4
TOK = N_FULL // T_SHARDS   # 2048 tokens per core
OUT = OUT_FULL // O_SHARDS  # 512 out features per core
KT = K // P                 # 16 contraction tiles
XB = 4                      # x blocks of 512 tokens
XBT = TOK // XB             # 512 tokens per block
OC = OUT // P               # 4 out-feature chunks of 128

F32 = mybir.dt.float32
BF16 = mybir.dt.bfloat16
BF16_NP = ml_dtypes.bfloat16

_CACHE = {}


def _build_nc():
    nc = bacc.Bacc(
        "TRN2",
        target_bir_lowering=False,
        debug=False,
        enable_asserts=False,
        num_devices=8,
    )
    x = nc.dram_tensor("x", [TOK, K], BF16, kind="ExternalInput").ap()
    mu = nc.dram_tensor("mu", [OUT, K], BF16, kind="ExternalInput").ap()
    rho = nc.dram_tensor("rho", [OUT, K], BF16, kind="ExternalInput").ap()
    eps = nc.dram_tensor("eps", [OUT, K], BF16, kind="ExternalInput").ap()
    # bias_pc[p, oc] = bias[oc*128 + p] (host pre-swizzled, per-partition)
    bias = nc.dram_tensor("bias", [P, OC], F32, kind="ExternalInput").ap()
    outT = nc.dram_tensor("outT", [OUT, TOK], BF16, kind="ExternalOutput").ap()

    ACT = mybir.ActivationFunctionType

    with tile.TileContext(nc) as tc:
        with (
            tc.tile_pool(name="const", bufs=1) as const,
            tc.tile_pool(name="wstage", bufs=1) as wstage,
            tc.tile_pool(name="xt", bufs=3) as xtp,
            tc.tile_pool(name="psum_mm", bufs=6, space="PSUM") as psum_mm,
            tc.tile_pool(name="outp", bufs=4) as outp,
        ):
            bias_sb = const.tile([P, OC], F32)
            nc.scalar.dma_start(bias_sb[:], bias)

            # Weight stage, chunked along out-features for pipelining.
            # XBAR transpose-load puts k on partitions: t[p, j, o] =
            # src[o, 128*j + p].  Chunk-major layout [P, OC, KT, P].
            # ALL XBAR transposes go on the sync ring (concurrent XBAR
            # activity on two HWDGE rings corrupts data); regular DMAs
            # (bias, output stores) go on the scalar ring.
            muT = wstage.tile([P, OC, KT, P], BF16, tag="muT")
            rhoT = wstage.tile([P, OC, KT, P], BF16, tag="rhoT")
            epsT = wstage.tile([P, OC, KT, P], BF16, tag="epsT")
            spT = wstage.tile([P, OC, KT, P], BF16, tag="spT")
            wT = wstage.tile([P, OC, KT, P], BF16, tag="wT")
            for oc in range(OC):
                osl = slice(oc * P, (oc + 1) * P)
                nc.sync.dma_start_transpose(rhoT[:, oc], rho[osl, :])
            exp_is = []
            for oc in range(OC):
                exp_is.append(
                    nc.scalar.activation(spT[:, oc], rhoT[:, oc], ACT.Exp)
                )
            for oc in range(OC):
                osl = slice(oc * P, (oc + 1) * P)
                nc.sync.dma_start_transpose(muT[:, oc], mu[osl, :])
                nc.sync.dma_start_transpose(epsT[:, oc], eps[osl, :])
            for oc in range(OC):
                # softplus(rho) = ln(exp(rho) + 1); rho <= ~0.5 so exp
                # cannot overflow.  Ordered after every Exp chunk so the
                # ACT function-table set switches only once.
                ln_i = nc.scalar.activation(
                    spT[:, oc], spT[:, oc], ACT.Ln, bias=1.0
                )
                add_dep_helper(
                    ln_i.ins,
                    exp_is[-1].ins,
                    sync=False,
                    reason="batch ACT table sets",
                )
                nc.vector.tensor_mul(spT[:, oc], spT[:, oc], epsT[:, oc])
                nc.vector.tensor_add(wT[:, oc], spT[:, oc], muT[:, oc])

            # Matmul stage: outT[o, t] = sum_k w[o, k] * x[t, k].
            for b in range(XB):
                tsl = slice(b * XBT, (b + 1) * XBT)
                xT = xtp.tile([P, KT, XBT], BF16)
                nc.sync.dma_start_transpose(xT[:], x[tsl, :])
                for oc in range(OC):
                    osl = slice(oc * P, (oc + 1) * P)
                    ps = psum_mm.tile([P, XBT], F32)
                    for j in range(KT):
                        nc.tensor.matmul(
                            ps[:],
                            lhsT=wT[:, oc, j, :],
                            rhs=xT[:, j, :],
                            start=(j == 0),
                            stop=(j == KT - 1),
                        )
                    ob = outp.tile([P, XBT], BF16)
                    nc.scalar.activation(
                        ob[:], ps[:], ACT.Identity, bias=bias_sb[:, oc : oc + 1]
                    )
                    nc.scalar.dma_start(outT[osl, tsl], ob[:])

    nc.compile()
    return nc


def _get_nc():
    if "nc" not in _CACHE:
        _CACHE["nc"] = _build_nc()
    return _CACHE["nc"]


def _make_in_maps(input, weight_mu, weight_rho, eps_weight, bias):
    in_maps = []
    for core in range(8):
        t, o = divmod(core, O_SHARDS)
        tsl = slice(t * TOK, (t + 1) * TOK)
        osl = slice(o * OUT, (o + 1) * OUT)
        bias_pc = np.ascontiguousarray(
            np.asarray(bias[osl], dtype=np.float32).reshape(OC, P).T
        )
        in_maps.append(
            {
                "x": np.ascontiguousarray(input[tsl, :].astype(BF16_NP)),
                "mu": np.ascontiguousarray(weight_mu[osl, :].astype(BF16_NP)),
                "rho": np.ascontiguousarray(weight_rho[osl, :].astype(BF16_NP)),
                "eps": np.ascontiguousarray(eps_weight[osl, :].astype(BF16_NP)),
                "bias": bias_pc,
            }
        )
    return in_maps


def run_sharded(input, weight_mu, weight_rho, eps_weight, bias, **run_kwargs):
    """Run the SPMD kernel; returns (full_output, BassKernelResults)."""
    nc = _get_nc()
    in_maps = _make_in_maps(input, weight_mu, weight_rho, eps_weight, bias)
    res = run_bass_kernel_spmd(nc, in_maps, list(range(8)), **run_kwargs)
    full = np.empty((N_FULL, OUT_FULL), dtype=np.float32)
    for core in range(8):
        t, o = divmod(core, O_SHARDS)
        full[t * TOK : (t + 1) * TOK, o * OUT : (o + 1) * OUT] = (
            res.results[core]["outT"].astype(np.float32).T
        )
    return full, res


def kernel(input, weight_mu, weight_rho, eps_weight, bias):
    full, _ = run_sharded(
        np.asarray(input),
        np.asarray(weight_mu),
        np.asarray(weight_rho),
        np.asarray(eps_weight),
        np.asarray(bias),
    )
    return full
